# revision 1
# baseline (speedup 1.0000x reference)
"""Graph-Transformer (3-layer) Trainium2 kernel v2, 8-core SPMD.

Structure per layer:
  LOOP1 (per window): batched kv+q indirect gathers, batched DVE score ops,
    PE one-hot aggregation, attention -> Wo -> FFN chain per window.
  LN batch (per window-half): LayerNorm over [P, half*D] tiles.
  LOOP2 (per window): next-layer q|k|v projection, table stores.
  AllGather of the kv table split in two halves, overlapped with compute.
Readout mean+MLP done on host from per-core partial sums.
"""
import sys

sys.path.insert(0, "/opt/trn_rl_repo")

import contextlib
import numpy as np
import ml_dtypes

import concourse.bass as bass
import concourse.tile as tile
from concourse import bacc, mybir
from concourse.bass_utils import run_bass_kernel_spmd
from concourse.masks import make_identity

P = 128
NCORES = 8
N, E, D, H, L = 50000, 800000, 128, 8, 3
HD = D // H
D2 = 2 * D
D3 = 3 * D
VOCAB = 7000
NL = N // NCORES            # 6250 nodes per core
NT = (NL + P - 1) // P      # 49 windows per core
NPAD = NT * P               # 6272 padded rows per core
LAST = NL - (NT - 1) * P    # 106 valid rows in last window
VT = (VOCAB + P - 1) // P   # 55
VPAD = VT * P               # 7040
H1W = 25                    # windows in half A
H2W = NT - H1W              # 24
NA = P * H1W                # 3200 rows per core, half A
NB = P * H2W                # 3072
GW = 2                      # windows per gather group
GATHER_MODE = "cols"        # "2dflat" | "cols"

F32 = mybir.dt.float32
BF16 = mybir.dt.bfloat16
FP8 = mybir.dt.float8e4
I32 = mybir.dt.int32
ADD = mybir.AluOpType.add
MUL = mybir.AluOpType.mult
SUB = mybir.AluOpType.subtract
AF = mybir.ActivationFunctionType
AX = mybir.AxisListType

_CACHE = {}


def _vrow(v):
    """vocab id -> permuted emb/embq/embkv row."""
    v = np.asarray(v, np.int64)
    return (v & 127) * VT + (v >> 7)


def _qrow(dl):
    """core-local node idx -> qloc row."""
    dl = np.asarray(dl, np.int64)
    return (dl & 127) * NT + (dl >> 7)


def _permrow(g):
    """global node id -> kvfull row (half-split, partition-major)."""
    g = np.asarray(g, np.int64)
    c = g // NL
    r = g % NL
    p = r & 127
    w = r >> 7
    return c * NPAD + p * NT + w


def _balance_nodes(edge_index):
    """Assign nodes to (core, window, slot) balancing in-edge counts per
    (core, window) bucket (LPT greedy). Returns node->core, node->dl,
    inv[c][r] = node."""
    import heapq
    e1 = np.asarray(edge_index[1], dtype=np.int64)
    deg = np.bincount(e1, minlength=N)
    order = np.argsort(-deg, kind="stable")
    nbuck = NCORES * NT
    cap = np.full(nbuck, P, np.int64)
    cap[NT - 1::NT] = LAST          # window 48 of each core holds 106
    heap = [(0, b) for b in range(nbuck)]
    heapq.heapify(heap)
    sums = np.zeros(nbuck, np.int64)
    fill = np.zeros(nbuck, np.int64)
    node_core = np.zeros(N, np.int64)
    node_dl = np.zeros(N, np.int64)
    inv = np.zeros((NCORES, NPAD), np.int64)
    for g in order:
        while True:
            s, b = heapq.heappop(heap)
            if s == sums[b] and fill[b] < cap[b]:
                break
        c, w = divmod(b, NT)
        p = fill[b]
        fill[b] += 1
        sums[b] += deg[g]
        if fill[b] < cap[b]:
            heapq.heappush(heap, (sums[b], b))
        node_core[g] = c
        node_dl[g] = w * P + p
        inv[c, w * P + p] = g
    return node_core, node_dl, inv


def _preprocess_edges(edge_index, x_idx, node_core, node_dl):
    e0 = np.asarray(edge_index[0], dtype=np.int64)
    e1 = np.asarray(edge_index[1], dtype=np.int64)
    core = node_core[e1]
    dl = node_dl[e1]
    w = dl >> 7
    cw = core * NT + w
    cnt = np.bincount(cw, minlength=NCORES * NT).reshape(NCORES, NT)
    K_w = np.maximum(1, -(-cnt.max(axis=0) // P))
    NBLK = int(K_w.sum())
    blk_start = np.concatenate([[0], np.cumsum(K_w)])[:-1]

    src = np.zeros((NCORES, NBLK * P), np.int64)
    dst_l = np.zeros((NCORES, NBLK * P), np.int64)
    dwin = np.full((NCORES, NBLK * P), 255, np.int16)
    for c in range(NCORES):
        m = core == c
        ec0, ew, edl = e0[m], w[m], dl[m]
        order = np.argsort(ew, kind="stable")
        ec0, ew, edl = ec0[order], ew[order], edl[order]
        cnts = cnt[c]
        run_start = np.concatenate([[0], np.cumsum(cnts)])[:-1]
        pos = np.arange(len(ec0)) - np.repeat(run_start, cnts)
        slot = np.repeat(blk_start * P, cnts) + pos
        src[c, slot] = ec0
        dst_l[c, slot] = edl
        dwin[c, slot] = (edl & 127).astype(np.int16)

    xi = np.asarray(x_idx, np.int64).reshape(N)

    def to_pt(a, dtype=np.int32):
        return np.ascontiguousarray(
            a.reshape(NCORES, NBLK, P).transpose(0, 2, 1)).astype(dtype)

    prow = node_core * NPAD + ((node_dl & 127) * NT + (node_dl >> 7))
    svocT = to_pt(xi[src])
    snodT = to_pt(prow[src])
    qvocT = to_pt(xi[dst_l + (np.arange(NCORES) * NL)[:, None]])
    qlocT = to_pt(_qrow(dst_l))

    eye = np.arange(P, dtype=np.int16)
    A = dwin.reshape(NCORES, NBLK, P)[:, :, :, None] == eye[None, None, None, :]
    # A8pack [NCORES, P(e), NBLK*P(n)]; ATpack [NCORES, P(n), NBLK*P(e)]
    A8 = np.ascontiguousarray(
        A.transpose(0, 2, 1, 3).reshape(NCORES, P, NBLK * P)
    ).astype(ml_dtypes.float8_e4m3fn)
    AT8 = np.ascontiguousarray(
        A.transpose(0, 3, 1, 2).reshape(NCORES, P, NBLK * P)
    ).astype(ml_dtypes.float8_e4m3fn)
    return K_w, NBLK, blk_start, svocT, snodT, qvocT, qlocT, A8, AT8


def _build_program(K_w, blk_start, NBLK, has_bias, has_ln_aff):
    nc = bacc.Bacc("TRN2", target_bir_lowering=False, debug=False,
                   num_devices=NCORES)
    dt = nc.dram_tensor
    tn = {}
    tn["embp"] = dt("embp", [VPAD, D], BF16, kind="ExternalInput")
    tn["ilocT"] = dt("ilocT", [P, NT], I32, kind="ExternalInput")
    tn["svocT"] = dt("svocT", [P, NBLK], I32, kind="ExternalInput")
    tn["snodT"] = dt("snodT", [P, NBLK], I32, kind="ExternalInput")
    tn["qvocT"] = dt("qvocT", [P, NBLK], I32, kind="ExternalInput")
    tn["qlocT"] = dt("qlocT", [P, NBLK], I32, kind="ExternalInput")
    tn["A8pack"] = dt("A8pack", [P, NBLK * P], FP8, kind="ExternalInput")
    tn["ATpack"] = dt("ATpack", [P, NBLK * P], FP8, kind="ExternalInput")
    tn["ilocraw"] = dt("ilocraw", [P, NT], I32, kind="ExternalInput")
    tn["Wqkv"] = dt("Wqkv", [L, D, D3], BF16, kind="ExternalInput")
    tn["Wo"] = dt("Wo", [L, D, D], BF16, kind="ExternalInput")
    tn["Wf1"] = dt("Wf1", [L, D, D2], BF16, kind="ExternalInput")
    tn["Wf2"] = dt("Wf2", [L, D2, D], BF16, kind="ExternalInput")
    tn["biasb"] = dt("biasb", [L, 4, P, D3], F32, kind="ExternalInput")
    tn["rowmask"] = dt("rowmask", [P, 1], F32, kind="ExternalInput")
    tn["out"] = dt("out", [1, D], F32, kind="ExternalOutput")

    with tile.TileContext(nc) as tc:
        _emit(nc, tc, tn, K_w, blk_start, NBLK, has_bias, has_ln_aff)
    nc.compile()
    return nc


def _emit(nc, tc, tn, K_w, blk_start, NBLK, has_bias, has_ln_aff):
    KGmax = max(int(sum(K_w[w0:min(w0 + GW, NT)]))
                for w0 in range(0, NT, GW))
    any_bias = any(has_bias)
    any_aff = any(has_ln_aff)

    ctx = contextlib.ExitStack()
    with ctx:
        stat = ctx.enter_context(tc.tile_pool(name="stat", bufs=1))
        kvp = ctx.enter_context(tc.tile_pool(name="kvp", bufs=2))
        qbp = ctx.enter_context(tc.tile_pool(name="qbp", bufs=2))
        sqp = ctx.enter_context(tc.tile_pool(name="sqp", bufs=2))
        awp = ctx.enter_context(tc.tile_pool(name="awp", bufs=2))
        smp = ctx.enter_context(tc.tile_pool(name="smp", bufs=3))
        sbw = ctx.enter_context(tc.tile_pool(name="sbw", bufs=3))
        pst = ctx.enter_context(tc.tile_pool(name="pst", bufs=2, space="PSUM"))
        psq = ctx.enter_context(tc.tile_pool(name="psq", bufs=2, space="PSUM"))
        psl = ctx.enter_context(tc.tile_pool(name="psl", bufs=2, space="PSUM"))
        psw = ctx.enter_context(tc.tile_pool(name="psw", bufs=2, space="PSUM"))
        dram = ctx.enter_context(tc.tile_pool(name="dram", bufs=1, space="DRAM"))
        dram2 = ctx.enter_context(tc.tile_pool(name="dram2", bufs=2,
                                               space="DRAM"))

        identb = stat.tile([P, P], BF16)
        make_identity(nc, identb[:])
        ones_col = stat.tile([P, 1], BF16)
        nc.gpsimd.memset(ones_col[:], 1.0)
        epsln = stat.tile([P, 1], F32)
        nc.gpsimd.memset(epsln[:], 1e-5)

        wqkv, wo, wf1, wf2 = [], [], [], []
        for l in range(L):
            t = stat.tile([D, D3], BF16, tag=f"wqkv{l}", name=f"wqkv{l}")
            nc.sync.dma_start(t[:], tn["Wqkv"][l])
            wqkv.append(t)
            t = stat.tile([D, D], BF16, tag=f"wo{l}", name=f"wo{l}")
            nc.sync.dma_start(t[:], tn["Wo"][l])
            wo.append(t)
            t = stat.tile([D, D2], BF16, tag=f"wf1{l}", name=f"wf1{l}")
            nc.sync.dma_start(t[:], tn["Wf1"][l])
            wf1.append(t)
            hs = []
            for h in range(2):
                t = stat.tile([D, D], BF16, tag=f"wf2{l}h{h}",
                              name=f"wf2{l}h{h}")
                nc.sync.dma_start(t[:], tn["Wf2"][l, h * D:(h + 1) * D])
                hs.append(t)
            wf2.append(hs)
        bias = []
        if any_bias or any_aff:
            for l in range(L):
                t = stat.tile([P, 4 * D3], F32, tag=f"bias{l}", name=f"bias{l}")
                nc.sync.dma_start(t[:].rearrange("p (n d) -> p n d", n=4),
                                  tn["biasb"][l].rearrange("n p d -> p n d"))
                bias.append(t)

        def bsl(l, i, off, width):
            return bias[l][:, i * D3 + off: i * D3 + off + width]

        idx_tiles = {}
        for nm in ["svocT", "snodT", "qvocT", "qlocT"]:
            t = stat.tile([P, NBLK], I32, tag=nm, name=nm)
            nc.sync.dma_start(t[:], tn[nm][:])
            idx_tiles[nm] = t
        ilocT = stat.tile([P, NT], I32)
        nc.sync.dma_start(ilocT[:], tn["ilocT"][:])
        ilocraw = stat.tile([P, NT], I32)
        nc.sync.dma_start(ilocraw[:], tn["ilocraw"][:])

        # big persistent state
        x_sb = stat.tile([P, NT * D], BF16)
        x_v = x_sb[:].rearrange("p (t d) -> p t d", t=NT)
        y_sb = stat.tile([P, NT * D], BF16)
        y_v = y_sb[:].rearrange("p (t d) -> p t d", t=NT)
        att_sb = stat.tile([P, NT * D], BF16)
        att_v = att_sb[:].rearrange("p (t d) -> p t d", t=NT)
        q_sb = stat.tile([P, NT * D], BF16)
        q_v = q_sb[:].rearrange("p (t d) -> p t d", t=NT)
        kv_sb = stat.tile([P, NT * D2], BF16)
        kv_v = kv_sb[:].rearrange("p (t d) -> p t d", t=NT)

        embq = dram.tile([VPAD, D], BF16)
        embkv = dram.tile([VPAD, D2], BF16)
        qloc = dram.tile([NPAD, D], BF16)
        kvloc = dram2.tile([NPAD, D2], BF16, tag="kvloc")

        # ---------- phase 0: vocab q/kv tables ----------
        with tc.tile_pool(name="ph0", bufs=1) as ph0:
            emb_sb = ph0.tile([P, VT * D], BF16)
            nc.sync.dma_start(emb_sb[:].rearrange("p (v d) -> p v d", v=VT),
                              tn["embp"][:].rearrange("(p v) d -> p v d", p=P))
            emb_vv = emb_sb[:].rearrange("p (v d) -> p v d", v=VT)
            for vt in range(VT):
                tp = pst.tile([P, P], BF16, space="PSUM", tag="tp", name="tp0")
                nc.tensor.transpose(tp[:], in_=emb_vv[:, vt, :],
                                    identity=identb[:])
                eT = sbw.tile([P, P], BF16, tag="eT", name="eT")
                nc.vector.tensor_copy(eT[:], tp[:])
                ps = psl.tile([P, D3], F32, space="PSUM", tag="lin",
                              name="qkv0")
                nc.tensor.matmul(ps[:], lhsT=eT[:], rhs=wqkv[0][:],
                                 start=True, stop=True)
                qkvt = sbw.tile([P, D3], BF16, tag="qkvt", name="qkvt")
                if has_bias[0]:
                    nc.vector.tensor_tensor(out=qkvt[:], in0=ps[:],
                                            in1=bsl(0, 0, 0, D3), op=ADD)
                else:
                    nc.scalar.activation(out=qkvt[:], in_=ps[:], func=AF.Copy)
                nc.sync.dma_start(embq[vt * P:(vt + 1) * P], qkvt[:, :D])
                nc.sync.dma_start(embkv[vt * P:(vt + 1) * P], qkvt[:, D:])
            # q0 gather (per window, from embq by raw vocab id)
            for t in range(NT):
                nc.gpsimd.indirect_dma_start(
                    out=q_v[:, t, :], out_offset=None, in_=embq[:],
                    in_offset=bass.IndirectOffsetOnAxis(
                        ap=ilocraw[:, t:t + 1], axis=0))
            # x0 gather
            if GATHER_MODE == "2dflat":
                nc.gpsimd.indirect_dma_start(
                    out=x_sb[:], out_offset=None, in_=tn["embp"][:],
                    in_offset=bass.IndirectOffsetOnAxis(ap=ilocT[:, :],
                                                        axis=0))
            else:
                for t in range(NT):
                    nc.gpsimd.indirect_dma_start(
                        out=x_v[:, t, :], out_offset=None, in_=tn["embp"][:],
                        in_offset=bass.IndirectOffsetOnAxis(
                            ap=ilocT[:, t:t + 1], axis=0))

        def gather(out_flat2d, out_3d, table, idxT, bs, KG, rowelems):
            if GATHER_MODE == "2dflat":
                nc.gpsimd.indirect_dma_start(
                    out=out_flat2d, out_offset=None, in_=table[:],
                    in_offset=bass.IndirectOffsetOnAxis(
                        ap=idxT[:, bs:bs + KG], axis=0))
            else:
                for j in range(KG):
                    nc.gpsimd.indirect_dma_start(
                        out=out_3d[:, j, :], out_offset=None, in_=table[:],
                        in_offset=bass.IndirectOffsetOnAxis(
                            ap=idxT[:, bs + j:bs + j + 1], axis=0))

        kvfull = None

        def layer_norm_batch(l, which, wa, wb, ysrc):
            """x_v[:, wa:wb] = LN(x_v[:, wa:wb] + ysrc[:, wa:wb]) [*g + b]."""
            M = wb - wa
            xh = x_v[:, wa:wb, :]
            nc.vector.tensor_tensor(out=xh, in0=xh, in1=ysrc[:, wa:wb, :],
                                    op=ADD)
            mu = smp.tile([P, NT], F32, tag="mu", name="mu")
            nc.vector.tensor_reduce(out=mu[:, :M], in_=xh, axis=AX.X, op=ADD)
            nc.vector.tensor_scalar_mul(mu[:, :M], mu[:, :M], 1.0 / D)
            # center in place
            nc.vector.tensor_tensor(
                out=xh, in0=xh, in1=mu[:, :M, None].to_broadcast([P, M, D]),
                op=SUB)
            var = smp.tile([P, NT], F32, tag="var", name="var")
            sqt = sqp.tile([P, H1W * D], BF16, tag="sqt", name="sqt")
            sqv = sqt[:, :M * D].rearrange("p (t d) -> p t d", t=M)
            nc.vector.tensor_tensor(out=sqv, in0=xh, in1=xh, op=MUL)
            nc.vector.tensor_reduce(out=var[:, :M], in_=sqv, axis=AX.X, op=ADD)
            std = smp.tile([P, NT], F32, tag="std", name="std")
            nc.scalar.activation(out=std[:, :M], in_=var[:, :M], func=AF.Sqrt,
                                 scale=1.0 / D, bias=epsln[:])
            rstd = smp.tile([P, NT], BF16, tag="rstd", name="rstd")
            with nc.allow_low_precision(reason="rstd scale factor"):
                nc.vector.reciprocal(rstd[:, :M], std[:, :M])
            nc.vector.tensor_tensor(
                out=xh, in0=xh, in1=rstd[:, :M, None].to_broadcast([P, M, D]),
                op=MUL)
            if has_ln_aff[l]:
                goff = 0 if which == 1 else D
                bi, boff = (2, D) if which == 1 else (3, D2)
                nc.vector.tensor_tensor(
                    out=xh, in0=xh,
                    in1=bsl(l, 3, goff, D)[:, None, :].to_broadcast([P, M, D]),
                    op=MUL)
                nc.vector.tensor_tensor(
                    out=xh, in0=xh,
                    in1=bsl(l, bi, boff, D)[:, None, :].to_broadcast(
                        [P, M, D]),
                    op=ADD)

        for l in range(L):
            if l == 0:
                kv_table, q_table = embkv, embq
                srcT = idx_tiles["svocT"]
                qidxT = idx_tiles["qvocT"]
            else:
                kv_table, q_table = kvfull, qloc
                srcT = idx_tiles["snodT"]
                qidxT = idx_tiles["qlocT"]

            for half, (wa, wb) in enumerate([(0, H1W), (H1W, NT)]):
                # ---- LOOP1 ----
                for w0 in range(wa, wb, GW):
                    gw = min(GW, wb - w0)
                    bs = int(blk_start[w0])
                    KG = int(sum(K_w[w0:w0 + gw]))
                    kvall = kvp.tile([P, KGmax * D2], BF16, tag="kvall",
                                     name="kvall")
                    kva = kvall[:, :KG * D2].rearrange("p (g d) -> p g d",
                                                       g=KG)
                    gather(kvall[:, :KG * D2], kva, kv_table, srcT, bs, KG, D2)
                    qball = qbp.tile([P, KGmax * D], BF16, tag="qball",
                                     name="qball")
                    qba = qball[:, :KG * D].rearrange("p (g d) -> p g d", g=KG)
                    aw = awp.tile([P, KGmax * P], FP8, tag="aw", name="aw")
                    nc.sync.dma_start(aw[:, :KG * P],
                                      tn["A8pack"][:, bs * P:(bs + KG) * P])
                    atw = awp.tile([P, KGmax * P], FP8, tag="atw", name="atw")
                    nc.sync.dma_start(atw[:, :KG * P],
                                      tn["ATpack"][:, bs * P:(bs + KG) * P])
                    qcum = 0
                    for w in range(w0, w0 + gw):
                        for j in range(int(K_w[w])):
                            col = qcum + j
                            qb_ps = psq.tile([P, D], F32, space="PSUM",
                                             tag="qb", name="qb")
                            nc.tensor.matmul(
                                qb_ps[:], lhsT=atw[:, col * P:(col + 1) * P],
                                rhs=q_v[:, w, :], start=True, stop=True)
                            nc.scalar.activation(out=qba[:, col, :],
                                                 in_=qb_ps[:], func=AF.Copy)
                        qcum += int(K_w[w])
                    # scores: tmul in place into qball
                    nc.vector.tensor_tensor(out=qba, in0=qba,
                                            in1=kva[:, :, :D], op=MUL)
                    s = smp.tile([P, KGmax * H], BF16, tag="s", name="s")
                    with nc.allow_low_precision(reason="16-elem score dots"):
                        nc.vector.tensor_reduce(
                            out=s[:, :KG * H],
                            in_=qball[:, :KG * D].rearrange("p (g d) -> p g d",
                                                            g=KG * H),
                            axis=AX.X, op=ADD)
                    w8 = smp.tile([P, KGmax * H], BF16, tag="w8", name="w8")
                    nc.scalar.activation(out=w8[:, :KG * H], in_=s[:, :KG * H],
                                         func=AF.Exp, scale=0.25)
                    # rhs in place in kvall cols 120:256 = [w8 | v*w]
                    nc.vector.tensor_tensor(
                        out=kva[:, :, D:].rearrange("p g (h d) -> p g h d",
                                                    h=H),
                        in0=kva[:, :, D:].rearrange("p g (h d) -> p g h d",
                                                    h=H),
                        in1=w8[:, :KG * H, None].rearrange(
                            "p (g h) d -> p g h d", g=KG
                        ).to_broadcast([P, KG, H, HD]),
                        op=MUL)
                    nc.vector.tensor_copy(
                        out=kva[:, :, 120:D],
                        in_=w8[:, :KG * H].rearrange("p (g h) -> p g h", g=KG))
                    cum = 0
                    for w in range(w0, w0 + gw):
                        Kw = int(K_w[w])
                        pw = psw.tile([P, 136], F32, space="PSUM", tag="pw",
                                      name="pw")
                        for j in range(Kw):
                            col = cum + j
                            nc.tensor.matmul(
                                pw[:], lhsT=aw[:, col * P:(col + 1) * P],
                                rhs=kva[:, col, 120:],
                                start=(j == 0), stop=(j == Kw - 1))
                        cum += Kw
                        # pw = [Z(8) | num(128)]
                        nc.scalar.activation(out=att_v[:, w, :],
                                             in_=pw[:, 8:], func=AF.Copy)
                        zr = smp.tile([P, 2 * H], F32, tag="zr", name="zr")
                        nc.vector.tensor_scalar_add(zr[:, :H], pw[:, :8],
                                                    1e-6)
                        nc.vector.reciprocal(zr[:, H:], zr[:, :H])
                        nc.vector.tensor_tensor(
                            out=att_v[:, w, :].rearrange("p (h d) -> p h d",
                                                         h=H),
                            in0=att_v[:, w, :].rearrange("p (h d) -> p h d",
                                                         h=H),
                            in1=zr[:, H:, None].to_broadcast([P, H, HD]),
                            op=MUL)
                        # y = attT @ Wo
                        tp = pst.tile([P, P], BF16, space="PSUM", tag="tp",
                                      name="tpa")
                        nc.tensor.transpose(tp[:], in_=att_v[:, w, :],
                                            identity=identb[:])
                        attT = sbw.tile([P, P], BF16, tag="attT", name="attT")
                        nc.scalar.activation(out=attT[:], in_=tp[:],
                                             func=AF.Copy)
                        y_ps = psl.tile([P, D3], F32, space="PSUM", tag="lin",
                                        name="y")
                        nc.tensor.matmul(y_ps[:, :D], lhsT=attT[:],
                                         rhs=wo[l][:], start=True, stop=True)
                        if has_bias[l]:
                            nc.vector.tensor_tensor(out=y_v[:, w, :],
                                                    in0=y_ps[:, :D],
                                                    in1=bsl(l, 1, 0, D),
                                                    op=ADD)
                        else:
                            nc.scalar.activation(out=y_v[:, w, :],
                                                 in_=y_ps[:, :D], func=AF.Copy)
                        # FFN(y) -> y2 parked in att_v
                        tp2 = pst.tile([P, P], BF16, space="PSUM", tag="tp",
                                       name="tpy")
                        nc.tensor.transpose(tp2[:], in_=y_v[:, w, :],
                                            identity=identb[:])
                        yT = sbw.tile([P, P], BF16, tag="yT", name="yT")
                        nc.scalar.activation(out=yT[:], in_=tp2[:],
                                             func=AF.Copy)
                        f1 = psl.tile([P, D3], F32, space="PSUM", tag="lin",
                                      name="f1")
                        nc.tensor.matmul(f1[:, :D2], lhsT=yT[:],
                                         rhs=wf1[l][:], start=True, stop=True)
                        h1 = sbw.tile([P, D2], BF16, tag="h1", name="h1")
                        if has_bias[l]:
                            nc.vector.tensor_tensor(out=h1[:], in0=f1[:, :D2],
                                                    in1=bsl(l, 1, D, D2),
                                                    op=ADD)
                            nc.vector.tensor_scalar_max(h1[:], h1[:], 0.0)
                        else:
                            nc.vector.tensor_scalar_max(h1[:], f1[:, :D2],
                                                        0.0)
                        tp3 = pst.tile([P, P], BF16, space="PSUM", tag="tp",
                                       name="tha")
                        nc.tensor.transpose(tp3[:], in_=h1[:, :D],
                                            identity=identb[:])
                        h1aT = sbw.tile([P, P], BF16, tag="h1aT", name="h1aT")
                        nc.vector.tensor_copy(h1aT[:], tp3[:])
                        tp4 = pst.tile([P, P], BF16, space="PSUM", tag="tp",
                                       name="thb")
                        nc.tensor.transpose(tp4[:], in_=h1[:, D:],
                                            identity=identb[:])
                        h1bT = sbw.tile([P, P], BF16, tag="h1bT", name="h1bT")
                        nc.vector.tensor_copy(h1bT[:], tp4[:])
                        f2 = psl.tile([P, D3], F32, space="PSUM", tag="lin",
                                      name="f2")
                        nc.tensor.matmul(f2[:, :D], lhsT=h1aT[:],
                                         rhs=wf2[l][0][:], start=True,
                                         stop=False)
                        nc.tensor.matmul(f2[:, :D], lhsT=h1bT[:],
                                         rhs=wf2[l][1][:], start=False,
                                         stop=True)
                        if has_bias[l]:
                            nc.vector.tensor_tensor(out=att_v[:, w, :],
                                                    in0=f2[:, :D],
                                                    in1=bsl(l, 2, 0, D),
                                                    op=ADD)
                        else:
                            nc.scalar.activation(out=att_v[:, w, :],
                                                 in_=f2[:, :D], func=AF.Copy)

                # ---- LN batch for this half ----
                layer_norm_batch(l, 1, wa, wb, y_v)     # x = LN(x + y)
                layer_norm_batch(l, 2, wa, wb, att_v)   # x = LN(x + y2)

                # ---- LOOP2: next-layer q/k/v ----
                if l < L - 1:
                    for w in range(wa, wb):
                        tp = pst.tile([P, P], BF16, space="PSUM", tag="tp",
                                      name="tpx")
                        nc.tensor.transpose(tp[:], in_=x_v[:, w, :],
                                            identity=identb[:])
                        xT = sbw.tile([P, P], BF16, tag="xT", name="xT")
                        nc.vector.tensor_copy(xT[:], tp[:])
                        qkv = psl.tile([P, D3], F32, space="PSUM", tag="lin",
                                       name="qkv")
                        nc.tensor.matmul(qkv[:], lhsT=xT[:],
                                         rhs=wqkv[l + 1][:], start=True,
                                         stop=True)
                        if has_bias[l + 1]:
                            nc.vector.tensor_tensor(out=q_v[:, w, :],
                                                    in0=qkv[:, :D],
                                                    in1=bsl(l + 1, 0, 0, D),
                                                    op=ADD)
                            nc.vector.tensor_tensor(out=kv_v[:, w, :],
                                                    in0=qkv[:, D:],
                                                    in1=bsl(l + 1, 0, D, D2),
                                                    op=ADD)
                        else:
                            nc.scalar.activation(out=q_v[:, w, :],
                                                 in_=qkv[:, :D], func=AF.Copy)
                            nc.scalar.activation(out=kv_v[:, w, :],
                                                 in_=qkv[:, D:], func=AF.Copy)
                    kvloc_v = kvloc[:].rearrange("(p w) d -> p w d", p=P)
                    if half == 0:
                        nc.sync.dma_start(kvloc_v[:, :H1W, :],
                                          kv_v[:, :H1W, :])
                    else:
                        nc.sync.dma_start(kvloc_v[:, H1W:, :],
                                          kv_v[:, H1W:, :])
            if l < L - 1:
                kvfull = dram2.tile([NCORES * NPAD, D2], BF16,
                                    tag="kvfull", name="kvfull",
                                    addr_space="Shared")
                nc.gpsimd.collective_compute(
                    "AllGather", mybir.AluOpType.bypass,
                    replica_groups=[list(range(NCORES))],
                    ins=[kvloc[:].opt()],
                    outs=[kvfull[:].opt()])

        # ---------- readout ----------
        rowmask = stat.tile([P, 1], F32)
        nc.sync.dma_start(rowmask[:], tn["rowmask"][:])
        nc.vector.tensor_tensor(out=x_v[:, NT - 1, :], in0=x_v[:, NT - 1, :],
                                in1=rowmask[:].to_broadcast([P, D]), op=MUL)
        acc = stat.tile([P, D], F32)
        nc.vector.tensor_reduce(
            out=acc[:], in_=x_sb[:].rearrange("p (w c) -> p c w", w=NT),
            axis=AX.X, op=ADD)
        accb = stat.tile([P, D], BF16)
        nc.vector.tensor_copy(accb[:], acc[:])
        o_ps = pst.tile([P, P], F32, space="PSUM", tag="tp", name="ops")
        nc.tensor.matmul(o_ps[:1, :D], lhsT=ones_col[:], rhs=accb[:],
                         start=True, stop=True)
        o_sb = sbw.tile([1, D], F32, tag="osb", name="osb")
        nc.vector.tensor_copy(o_sb[:], o_ps[:1, :D])
        nc.sync.dma_start(tn["out"][:], o_sb[:])


def kernel(**inputs):
    x_idx = np.asarray(inputs["x_idx"]).reshape(N).astype(np.int64)
    edge_index = np.asarray(inputs["edge_index"])
    getf = lambda k: np.asarray(inputs[k], np.float32)
    emb = getf("emb")
    bq, bk, bv, bo = getf("bq"), getf("bk"), getf("bv"), getf("bo")
    bf1, bf2 = getf("bf1"), getf("bf2")
    g1, be1, g2, be2 = getf("g1"), getf("be1"), getf("g2"), getf("be2")

    node_core, node_dl, inv = _balance_nodes(edge_index)
    (K_w, NBLK, blk_start, svocT, snodT, qvocT, qlocT, A8, AT8) = \
        _preprocess_edges(edge_index, x_idx, node_core, node_dl)
    has_bias = [bool(np.any(bq[l]) or np.any(bk[l]) or np.any(bv[l])
                     or np.any(bo[l]) or np.any(bf1[l]) or np.any(bf2[l]))
                for l in range(L)]
    has_ln_aff = [bool(np.any(g1[l] != 1) or np.any(be1[l])
                       or np.any(g2[l] != 1) or np.any(be2[l]))
                  for l in range(L)]

    prog_key = (tuple(K_w.tolist()), tuple(has_bias), tuple(has_ln_aff))
    if prog_key not in _CACHE:
        _CACHE[prog_key] = _build_program(K_w, blk_start, NBLK,
                                          has_bias, has_ln_aff)
    nc = _CACHE[prog_key]

    emb_pad = np.zeros((VPAD, D), np.float32)
    emb_pad[_vrow(np.arange(VOCAB))] = emb
    embp = emb_pad.astype(ml_dtypes.bfloat16)

    bias_blob = np.zeros((L, 4, P, D3), np.float32)
    for l in range(L):
        bias_blob[l, 0, :, :D] = bq[l]
        bias_blob[l, 0, :, D:D2] = bk[l]
        bias_blob[l, 0, :, D2:] = bv[l]
        bias_blob[l, 1, :, :D] = bo[l]
        bias_blob[l, 1, :, D:] = bf1[l]
        bias_blob[l, 2, :, :D] = bf2[l]
        bias_blob[l, 2, :, D:D2] = be1[l]
        bias_blob[l, 3, :, :D] = g1[l]
        bias_blob[l, 3, :, D:D2] = g2[l]
        bias_blob[l, 3, :, D2:] = be2[l]

    iloc = np.zeros((NCORES, P, NT), np.int32)
    ilocr = np.zeros((NCORES, P, NT), np.int32)
    for c in range(NCORES):
        pad = x_idx[inv[c]]
        padmask = np.ones(NPAD, bool)
        pm = padmask.reshape(NT, P)
        pm[NT - 1, LAST:] = False
        pad = np.where(padmask, pad, 0)
        iloc[c] = _vrow(pad).reshape(NT, P).T.astype(np.int32)
        ilocr[c] = pad.reshape(NT, P).T.astype(np.int32)

    tobf = lambda a: np.ascontiguousarray(a).astype(ml_dtypes.bfloat16)
    shared = {
        "embp": embp,
        "biasb": bias_blob,
        "Wqkv": tobf(np.concatenate(
            [getf("Wq"), getf("Wk"), getf("Wv")], axis=2)),
        "Wo": tobf(getf("Wo")),
        "Wf1": tobf(getf("Wf1")),
        "Wf2": tobf(getf("Wf2")),
    }
    rowmask = np.zeros((P, 1), np.float32)
    rowmask[:LAST] = 1.0
    in_maps = []
    for c in range(NCORES):
        m = dict(shared)
        m["ilocT"] = np.ascontiguousarray(iloc[c])
        m["rowmask"] = rowmask
        m["svocT"] = svocT[c]
        m["snodT"] = snodT[c]
        m["qvocT"] = qvocT[c]
        m["qlocT"] = qlocT[c]
        m["A8pack"] = A8[c]
        m["ATpack"] = AT8[c]
        m["ilocraw"] = np.ascontiguousarray(ilocr[c])
        in_maps.append(m)

    kernel.last_nc = nc
    kernel.last_in_maps = in_maps
    res = run_bass_kernel_spmd(nc, in_maps, list(range(NCORES)),
                               **getattr(kernel, "run_kwargs", {}))
    kernel.last_results = res
    total = np.zeros((1, D), np.float32)
    for c in range(NCORES):
        total += res.results[c]["out"]
    xm = total / N
    o = np.maximum(xm @ getf("mW0") + getf("mb0"), 0.0)
    o = np.maximum(o @ getf("mW1") + getf("mb1"), 0.0)
    return (o @ getf("mW2") + getf("mb2")).astype(np.float32)

